# revision 27
# baseline (speedup 1.0000x reference)
"""Builder + host glue for the ViT attention kernel on 8 trn2 cores.

Reference computation (per batch b):
    qkv = x @ w_qkv.T ; q,k,v split; per head: softmax(q k^T / sqrt(dh)) v
    out = attn @ w_out.T + b_out

Sharding: data-parallel over batch (8 batches per core).

Structure (v3): chunk-outer pipeline. T=1576 tokens split into 4 chunks
of 394 (2 batches each). Inputs arrive as a few wide host-packed DMAs
spread over 4 queues. Per chunk: QK projection in (q_j, k_j) pair-block
order; the previous chunk's V-projection / attention pairs / out-
projection run as fillers interleaved into the QK matmuls. Attention
uses a ones-column (placed FIRST) in the V tiles so softmax denominators
fall out of the AV matmul at psum row 0, where the DVE reciprocal can
read them directly (no copy).
"""

from collections import deque

import numpy as np
import ml_dtypes

import concourse.bass as bass
import concourse.tile as tile
from concourse import bacc, mybir
from concourse.bass_utils import run_bass_kernel_spmd

P = 128
B, N, D = 64, 197, 768
H, DH = 12, 64
NCORES = 8
BPC = B // NCORES          # 8 batches per core
T = BPC * N                # 1576 tokens per core
KT = D // P                # 6 contraction tiles
NPAIR = H // 2             # 6 head pairs
SCALE = DH ** -0.5
N2 = 2 * N                 # 394 (one chunk = 2 batches)
NCHUNK = 4
JT1 = N - P                # 69: second j-tile size
VB = 128                   # v block width per head: [ones, 63 pad, 64 dims]
VD = 64                    # dims offset within a v block (aligned psum access)
VW = VB * H                # 1536: v columns

BF = mybir.dt.bfloat16
F32 = mybir.dt.float32
EXP = mybir.ActivationFunctionType.Exp
IDENT = mybir.ActivationFunctionType.Identity


def build_nc():
    nc = bacc.Bacc(
        "TRN2", target_bir_lowering=False, debug=False, num_devices=NCORES
    )
    # host-packed wide inputs (see host_in_maps for layouts)
    xP = nc.dram_tensor("xP", [P, KT * T], BF, kind="ExternalInput").ap()
    wqkP = nc.dram_tensor("wqkP", [P, NPAIR * KT * 256], BF, kind="ExternalInput").ap()
    wvP = nc.dram_tensor("wvP", [P, KT * D], BF, kind="ExternalInput").ap()
    woP = nc.dram_tensor("woP", [P, KT * D], BF, kind="ExternalInput").ap()
    bias = nc.dram_tensor("bias", [P, KT], F32, kind="ExternalInput").ap()
    outT = nc.dram_tensor("outT", [D, T], F32, kind="ExternalOutput").ap()

    with tile.TileContext(nc) as tc:
        with (
            tc.tile_pool(name="big", bufs=1) as big,
            tc.tile_pool(name="exp", bufs=10) as sb_exp,
            tc.tile_pool(name="rec", bufs=6) as sb_rec,
            tc.tile_pool(name="bsb", bufs=6) as sb_bsb,
            tc.tile_pool(name="osb", bufs=4) as sb_osb,
            tc.tile_pool(name="ps_pj", bufs=3, space="PSUM") as ps_pj,
            tc.tile_pool(name="ps_sc", bufs=3, space="PSUM") as ps_sc,
            tc.tile_pool(name="ps_o", bufs=2, space="PSUM") as ps_o,
        ):
            # ---- persistent buffers -------------------------------------
            bias_sb = big.tile([P, KT], F32, tag="bias")

            x_sb = [
                big.tile([P, KT * N2], BF, tag=f"x{c}", name=f"x{c}")
                for c in range(NCHUNK)
            ]
            wqk_sb = [
                big.tile([P, KT * 256], BF, tag=f"wqk{j}", name=f"wqk{j}")
                for j in range(NPAIR)
            ]
            wv_sb = big.tile([P, KT * D], BF, tag="wv", name="wv")
            wo_sb = big.tile([P, KT * D], BF, tag="wo", name="wo")

            # qk_sb[m][c]: m<6 -> q head-pair m ; m>=6 -> k head-pair m-6.
            # layout [e within pair (2 heads x 64), t within chunk c]
            qk_sb = [
                [
                    big.tile([P, N2], BF, tag=f"qk{m}_{c}", name=f"qk{m}_{c}")
                    for c in range(NCHUNK)
                ]
                for m in range(2 * NPAIR)
            ]
            # v tiles per (batch, j-tile): [j, 12*96] blocks of
            # [ones, 31 pad, 64 dims] so AV psum row 0 = denominators
            # (readable by the custom-DVE reciprocal) and dims land
            # 32-aligned for the normalize muls.
            v_sb = [
                big.tile([P, VW], BF, tag=f"v{i}", name=f"v{i}")
                for i in range(2 * BPC)
            ]
            for i in range(2 * BPC):
                ones_cols = v_sb[i][:].rearrange("p (h c) -> p h c", c=VB)[
                    :, :, 0:1
                ]
                nc.gpsimd.memset(ones_cols, 1.0)
            # attention output, [e, t] layout, tiles per (pair, batch-pair)
            at_sb = [
                [
                    big.tile([P, N2], BF, tag=f"at{p}_{b2}", name=f"at{p}_{b2}")
                    for b2 in range(BPC // 2)
                ]
                for p in range(NPAIR)
            ]

            # ---- input DMAs (few wide transfers, spread over queues) ----
            HKN = KT * N2 // 2  # 1182: half-chunk cols
            # critical first chunk split across two queues
            nc.scalar.dma_start(x_sb[0][:, 0:HKN], xP[:, 0:HKN])
            nc.gpsimd.dma_start(x_sb[0][:, HKN : 2 * HKN], xP[:, HKN : 2 * HKN])
            for j in range(NPAIR):
                eng = nc.sync if j % 2 == 0 else nc.scalar
                eng.dma_start(
                    wqk_sb[j][:], wqkP[:, j * KT * 256 : (j + 1) * KT * 256]
                )
            nc.scalar.dma_start(x_sb[1][:], xP[:, KT * N2 : 2 * KT * N2])
            nc.gpsimd.dma_start(wv_sb[:], wvP)
            nc.gpsimd.dma_start(x_sb[2][:], xP[:, 2 * KT * N2 : 3 * KT * N2])
            nc.scalar.dma_start(x_sb[3][:], xP[:, 3 * KT * N2 : 4 * KT * N2])
            nc.gpsimd.dma_start(wo_sb[:], woP)
            nc.scalar.dma_start(bias_sb[:], bias)

            def x_ap(c, k, off=0, ln=N2):
                return x_sb[c][:, k * N2 + off : k * N2 + off + ln]

            def wqk_ap(j, k, half):
                c0 = k * 256 + half * P
                return wqk_sb[j][:, c0 : c0 + P]

            # ---- QK projection unit: one (chunk, j, q/k) psum group -----
            def qk_unit(c, j, half):
                m = j + NPAIR * half
                psum = ps_pj.tile([P, 512], F32, tag="pj", name="pjqk")[:, :N2]
                for k in range(KT):
                    nc.tensor.matmul(
                        psum,
                        wqk_ap(j, k, half),
                        x_ap(c, k),
                        start=(k == 0),
                        stop=(k == KT - 1),
                    )
                nc.vector.tensor_copy(out=qk_sb[m][c][:], in_=psum)

            # ---- V projection units -------------------------------------
            def vproj_unit(b, jt, c0, cl):
                def emit():
                    c = b // 2
                    off = (b % 2) * N + jt * P
                    rl = P if jt == 0 else JT1
                    i = 2 * b + jt
                    psum = ps_pj.tile([P, 512], F32, tag="pj", name="pjv")[:rl, :cl]
                    for k in range(KT):
                        nc.tensor.matmul(
                            psum,
                            x_ap(c, k, off, rl),
                            wv_sb[:, k * D + c0 : k * D + c0 + cl],
                            start=(k == 0),
                            stop=(k == KT - 1),
                        )
                    hs = c0 // DH
                    nh = cl // DH
                    out_ap = v_sb[i][
                        :rl, VB * hs : VB * (hs + nh)
                    ].rearrange("p (h c) -> p h c", c=VB)[:, :, VD : VD + DH]
                    nc.scalar.copy(
                        out=out_ap,
                        in_=psum.rearrange("p (h c) -> p h c", c=DH),
                    )

                return emit

            def vproj_units(b):
                # g0 units (heads 0-7) first so early pairs unblock sooner
                return [
                    vproj_unit(b, jt, c0, cl)
                    for c0, cl in ((0, 512), (512, 256))
                    for jt in range(2)
                ]

            # ---- out-projection units -----------------------------------
            op_cnt = [0]

            def outproj_unit(b2, m, half=None):
                def emit():
                    o0 = 0 if half is None else half * N
                    tl = N2 if half is None else N
                    t0 = b2 * N2 + o0
                    psum = ps_pj.tile([P, 512], F32, tag="pj", name="pjo")[:, :tl]
                    for k in range(KT):
                        nc.tensor.matmul(
                            psum,
                            wo_sb[:, k * D + m * P : k * D + (m + 1) * P],
                            at_sb[k][b2][:, o0 : o0 + tl],
                            start=(k == 0),
                            stop=(k == KT - 1),
                        )
                    osb = sb_osb.tile([P, 512], F32, tag="osb", name="osb")[:, :tl]
                    if op_cnt[0] % 2 == 0:
                        nc.scalar.activation(
                            osb, psum, IDENT, bias=bias_sb[:, m : m + 1]
                        )
                    else:
                        nc.vector.tensor_scalar_add(
                            out=osb, in0=psum, scalar1=bias_sb[:, m : m + 1]
                        )
                    eng = (nc.sync, nc.gpsimd)[op_cnt[0] % 2]
                    op_cnt[0] += 1
                    eng.dma_start(outT[m * P : (m + 1) * P, t0 : t0 + tl], osb)

                return emit

            # ---- one attention head-pair --------------------------------
            def pair_unit(b, p):
                def emit():
                    c = b // 2
                    tb = (b % 2) * N
                    qT = qk_sb[p][c]
                    kTt = qk_sb[NPAIR + p][c]
                    expT = []
                    for h in (0, 1):
                        e0 = DH * h
                        ps_s = ps_sc.tile([P, N2], F32, tag="sc", name="sc")
                        nc.tensor.matmul(
                            ps_s[0:P, 0:N],
                            kTt[e0 : e0 + DH, tb : tb + P],
                            qT[e0 : e0 + DH, tb : tb + N],
                            start=True,
                            stop=True,
                            tile_position=(e0, 0),
                        )
                        nc.tensor.matmul(
                            ps_s[0:JT1, N:N2],
                            kTt[e0 : e0 + DH, tb + P : tb + N],
                            qT[e0 : e0 + DH, tb : tb + N],
                            start=True,
                            stop=True,
                            tile_position=(e0, 0),
                        )
                        e = sb_exp.tile([P, N2], BF, tag="expT", name="expT")
                        nc.scalar.activation(e[:], ps_s[:], EXP)
                        expT.append(e)
                    # pso rows: 0 = denominators (ones col), 32..95 = out dims
                    pso = ps_o.tile([VB, N2], F32, tag="o", name="o")
                    v0, v1 = v_sb[2 * b], v_sb[2 * b + 1]
                    for h in (0, 1):
                        g = 2 * p + h
                        vc = VB * g
                        nc.tensor.matmul(
                            pso[:, N * h : N * h + N],
                            v0[0:P, vc : vc + VB],
                            expT[h][0:P, 0:N],
                            start=True,
                            stop=False,
                        )
                        nc.tensor.matmul(
                            pso[:, N * h : N * h + N],
                            v1[0:JT1, vc : vc + VB],
                            expT[h][0:JT1, N:N2],
                            start=False,
                            stop=True,
                        )
                    # approx reciprocal straight off psum row 0
                    rec = sb_rec.tile([1, N2], F32, tag="rec", name="rec")
                    nc.vector.reciprocal_approx_fast(out=rec[:], in_=pso[0:1, :])
                    bsb = sb_bsb.tile([DH, N2], F32, tag="bsb", name="bsb")
                    nc.gpsimd.partition_broadcast(bsb[:], rec[:])
                    for h in (0, 1):
                        nc.vector.tensor_mul(
                            out=at_sb[p][b // 2][
                                DH * h : DH * h + DH, N * (b % 2) : N * (b % 2) + N
                            ],
                            in0=pso[VD : VD + DH, N * h : N * h + N],
                            in1=bsb[:, N * h : N * h + N],
                        )

                return emit

            # ---- driver: chunk-outer pipeline ---------------------------
            # HAM discipline: attention pairs are Scalar-bound and leave the
            # PE half-idle, so never emit more than one pair between long
            # (projection) matmul units; out-projection lags 2 chunks so the
            # final drain still has long units to interleave 1:1.
            pairs_q = deque()
            for c in range(NCHUNK):
                longs = []
                for j in range(NPAIR):
                    longs.append(lambda c=c, j=j: qk_unit(c, j, 0))
                    longs.append(lambda c=c, j=j: qk_unit(c, j, 1))
                if c < NCHUNK - 1:
                    longs += vproj_units(2 * c) + vproj_units(2 * c + 1)
                if c >= 2:
                    longs += [outproj_unit(c - 2, m) for m in range(KT)]
                np_, nl = len(pairs_q), len(longs)
                done = 0
                for i, L in enumerate(longs):
                    L()
                    while pairs_q and (done + 1) * nl <= (i + 1) * np_:
                        pairs_q.popleft()()
                        done += 1
                while pairs_q:
                    pairs_q.popleft()()
                if c < NCHUNK - 1:
                    for j in range(NPAIR):
                        pairs_q.append(pair_unit(2 * c, j))
                        pairs_q.append(pair_unit(2 * c + 1, j))
            # drain: batch-6/7 pairs strictly 1:1 with remaining long units
            # (vproj(6,7), outproj(2), outproj(3) batch halves) so the PE
            # never idles long enough to re-throttle
            vp6 = vproj_units(6)
            vp7 = vproj_units(7)
            p6 = [pair_unit(6, j) for j in range(NPAIR)]
            p7 = [pair_unit(7, j) for j in range(NPAIR)]
            op2 = [outproj_unit(2, m) for m in range(KT)]
            seq = [
                vp6[0], vp6[1],
                p6[0], vp6[2], p6[1], vp6[3],
                p6[2], vp7[0], p6[3], vp7[1],
                p6[4], op2[0], p6[5], op2[1],
                p7[0], op2[2], p7[1], op2[3],
                p7[2], vp7[2], p7[3], vp7[3],
                p7[4], op2[4], p7[5], op2[5],
            ]
            seq += [outproj_unit(3, m, half=0) for m in range(KT)]
            seq += [outproj_unit(3, m, half=1) for m in range(KT)]
            for u in seq:
                u()

    nc.compile()
    return nc


def host_in_maps(x, w_qkv, w_out, b_out):
    """Full fp32 inputs -> list of 8 per-core input dicts (bf16, packed)."""
    bf16 = ml_dtypes.bfloat16
    wq = w_qkv[0:D] * SCALE
    wk = w_qkv[D : 2 * D]
    wv = w_qkv[2 * D : 3 * D]
    wqkT = np.concatenate([wq, wk], axis=0).T  # [768 d, 1536 e]
    # wqkP[p, j*1536 + k*256 + half*128 + col] = wqkT[k*128+p, 768*half + j*128 + col]
    wqkP = np.ascontiguousarray(
        wqkT.reshape(KT, P, 2, NPAIR, P).transpose(1, 3, 0, 2, 4).reshape(P, -1)
    ).astype(bf16)
    # w[v,o]P[p, k*768 + col] = wT[k*128+p, col]
    wvP = np.ascontiguousarray(
        wv.T.reshape(KT, P, D).transpose(1, 0, 2).reshape(P, -1)
    ).astype(bf16)
    woP = np.ascontiguousarray(
        w_out.T.reshape(KT, P, D).transpose(1, 0, 2).reshape(P, -1)
    ).astype(bf16)
    bias_ = np.ascontiguousarray(b_out.reshape(KT, P).T).astype(np.float32)
    in_maps = []
    for core in range(NCORES):
        xc = x[core * BPC : (core + 1) * BPC].reshape(T, D)
        # xP[p, c*2364 + k*394 + t'] = xc[c*394 + t', k*128 + p]
        xP = np.ascontiguousarray(
            xc.reshape(NCHUNK, N2, KT, P).transpose(3, 0, 2, 1).reshape(P, -1)
        ).astype(bf16)
        in_maps.append(
            {"xP": xP, "wqkP": wqkP, "wvP": wvP, "woP": woP, "bias": bias_}
        )
    return in_maps


def host_gather(results):
    """8 per-core {outT: [768, 1576] fp32} -> full [64, 197, 768] fp32."""
    out = np.empty((B, N, D), dtype=np.float32)
    for c in range(NCORES):
        oc = results[c]["outT"]  # [D, T]
        out[c * BPC : (c + 1) * BPC] = oc.T.reshape(BPC, N, D)
    return out


_NC_CACHE = []


def kernel(x, w_qkv, w_out, b_out):
    """Full-input entry point: shards batch over 8 NeuronCores, runs the
    Bass kernel, gathers the full [64, 197, 768] fp32 output."""
    if not _NC_CACHE:
        _NC_CACHE.append(build_nc())
    nc = _NC_CACHE[0]
    in_maps = host_in_maps(
        np.asarray(x, dtype=np.float32),
        np.asarray(w_qkv, dtype=np.float32),
        np.asarray(w_out, dtype=np.float32),
        np.asarray(b_out, dtype=np.float32),
    )
    res = run_bass_kernel_spmd(nc, in_maps, core_ids=list(range(NCORES)))
    return host_gather(res.results)


# revision 28
# speedup vs baseline: 1.1808x; 1.1808x over previous
"""Builder + host glue for the ViT attention kernel on 8 trn2 cores.

Reference computation (per batch b):
    qkv = x @ w_qkv.T ; q,k,v split; per head: softmax(q k^T / sqrt(dh)) v
    out = attn @ w_out.T + b_out

Sharding: data-parallel over batch (8 batches per core).

Structure (v3): chunk-outer pipeline. T=1576 tokens split into 4 chunks
of 394 (2 batches each). Inputs arrive as a few wide host-packed DMAs
spread over 4 queues. Per chunk: QK projection in (q_j, k_j) pair-block
order; the previous chunk's V-projection / attention pairs / out-
projection run as fillers interleaved into the QK matmuls. Attention
uses a ones-column (placed FIRST) in the V tiles so softmax denominators
fall out of the AV matmul at psum row 0, where the DVE reciprocal can
read them directly (no copy).
"""

from collections import deque

import numpy as np
import ml_dtypes

import concourse.bass as bass
import concourse.tile as tile
from concourse import bacc, mybir
from concourse.bass_utils import run_bass_kernel_spmd

P = 128
B, N, D = 64, 197, 768
H, DH = 12, 64
NCORES = 8
BPC = B // NCORES          # 8 batches per core
T = BPC * N                # 1576 tokens per core
KT = D // P                # 6 contraction tiles
NPAIR = H // 2             # 6 head pairs
SCALE = DH ** -0.5
N2 = 2 * N                 # 394 (one chunk = 2 batches)
NCHUNK = 4
JT1 = N - P                # 69: second j-tile size
VB = 128                   # v block width per head: [ones, 63 pad, 64 dims]
VD = 64                    # dims offset within a v block (aligned psum access)
VW = VB * H                # 1536: v columns

BF = mybir.dt.bfloat16
F32 = mybir.dt.float32
EXP = mybir.ActivationFunctionType.Exp
IDENT = mybir.ActivationFunctionType.Identity


def build_nc():
    nc = bacc.Bacc(
        "TRN2", target_bir_lowering=False, debug=False, num_devices=NCORES
    )
    # host-packed wide inputs (see host_in_maps for layouts)
    xP = nc.dram_tensor("xP", [P, KT * T], BF, kind="ExternalInput").ap()
    wqkP = nc.dram_tensor("wqkP", [P, NPAIR * KT * 256], BF, kind="ExternalInput").ap()
    wvP = nc.dram_tensor("wvP", [P, KT * D], BF, kind="ExternalInput").ap()
    woP = nc.dram_tensor("woP", [P, KT * D], BF, kind="ExternalInput").ap()
    bias = nc.dram_tensor("bias", [P, KT], F32, kind="ExternalInput").ap()
    outT = nc.dram_tensor("outT", [D, T], F32, kind="ExternalOutput").ap()

    with tile.TileContext(nc) as tc:
        with (
            tc.tile_pool(name="big", bufs=1) as big,
            tc.tile_pool(name="exp", bufs=10) as sb_exp,
            tc.tile_pool(name="rec", bufs=6) as sb_rec,
            tc.tile_pool(name="bsb", bufs=6) as sb_bsb,
            tc.tile_pool(name="osb", bufs=4) as sb_osb,
            tc.tile_pool(name="ps_pj", bufs=3, space="PSUM") as ps_pj,
            tc.tile_pool(name="ps_sc", bufs=3, space="PSUM") as ps_sc,
            tc.tile_pool(name="ps_o", bufs=2, space="PSUM") as ps_o,
        ):
            # ---- persistent buffers -------------------------------------
            bias_sb = big.tile([P, KT], F32, tag="bias")

            x_sb = [
                big.tile([P, KT * N2], BF, tag=f"x{c}", name=f"x{c}")
                for c in range(NCHUNK)
            ]
            wqk_sb = [
                big.tile([P, KT * 256], BF, tag=f"wqk{j}", name=f"wqk{j}")
                for j in range(NPAIR)
            ]
            wv_sb = big.tile([P, KT * D], BF, tag="wv", name="wv")
            wo_sb = big.tile([P, KT * D], BF, tag="wo", name="wo")

            # qk_sb[m][c]: m<6 -> q head-pair m ; m>=6 -> k head-pair m-6.
            # layout [e within pair (2 heads x 64), t within chunk c]
            qk_sb = [
                [
                    big.tile([P, N2], BF, tag=f"qk{m}_{c}", name=f"qk{m}_{c}")
                    for c in range(NCHUNK)
                ]
                for m in range(2 * NPAIR)
            ]
            # v tiles per (batch, j-tile): [j, 12*96] blocks of
            # [ones, 31 pad, 64 dims] so AV psum row 0 = denominators
            # (readable by the custom-DVE reciprocal) and dims land
            # 32-aligned for the normalize muls.
            v_sb = [
                big.tile([P, VW], BF, tag=f"v{i}", name=f"v{i}")
                for i in range(2 * BPC)
            ]
            for i in range(2 * BPC):
                ones_cols = v_sb[i][:].rearrange("p (h c) -> p h c", c=VB)[
                    :, :, 0:1
                ]
                nc.gpsimd.memset(ones_cols, 1.0)
            # attention output, [e, t] layout, tiles per (pair, batch-pair)
            at_sb = [
                [
                    big.tile([P, N2], BF, tag=f"at{p}_{b2}", name=f"at{p}_{b2}")
                    for b2 in range(BPC // 2)
                ]
                for p in range(NPAIR)
            ]

            # ---- input DMAs (few wide transfers, spread over queues) ----
            HKN = KT * N2 // 2  # 1182: half-chunk cols
            # critical first chunk split across two queues
            nc.scalar.dma_start(x_sb[0][:, 0:HKN], xP[:, 0:HKN])
            nc.gpsimd.dma_start(x_sb[0][:, HKN : 2 * HKN], xP[:, HKN : 2 * HKN])
            for j in range(NPAIR):
                eng = nc.sync if j % 2 == 0 else nc.scalar
                eng.dma_start(
                    wqk_sb[j][:], wqkP[:, j * KT * 256 : (j + 1) * KT * 256]
                )
            nc.scalar.dma_start(x_sb[1][:], xP[:, KT * N2 : 2 * KT * N2])
            nc.gpsimd.dma_start(wv_sb[:], wvP)
            nc.gpsimd.dma_start(x_sb[2][:], xP[:, 2 * KT * N2 : 3 * KT * N2])
            nc.scalar.dma_start(x_sb[3][:], xP[:, 3 * KT * N2 : 4 * KT * N2])
            nc.gpsimd.dma_start(wo_sb[:], woP)
            nc.scalar.dma_start(bias_sb[:], bias)

            def x_ap(c, k, off=0, ln=N2):
                return x_sb[c][:, k * N2 + off : k * N2 + off + ln]

            def wqk_ap(j, k, half):
                c0 = k * 256 + half * P
                return wqk_sb[j][:, c0 : c0 + P]

            # ---- QK projection unit: one (chunk, j, q/k) psum group -----
            def qk_unit(c, j, half):
                m = j + NPAIR * half
                psum = ps_pj.tile([P, 512], F32, tag="pj", name="pjqk")[:, :N2]
                for k in range(KT):
                    nc.tensor.matmul(
                        psum,
                        wqk_ap(j, k, half),
                        x_ap(c, k),
                        start=(k == 0),
                        stop=(k == KT - 1),
                    )
                nc.vector.tensor_copy(out=qk_sb[m][c][:], in_=psum)

            # ---- V projection units -------------------------------------
            def vproj_unit(b, jt, c0, cl):
                def emit():
                    c = b // 2
                    off = (b % 2) * N + jt * P
                    rl = P if jt == 0 else JT1
                    i = 2 * b + jt
                    psum = ps_pj.tile([P, 512], F32, tag="pj", name="pjv")[:rl, :cl]
                    for k in range(KT):
                        nc.tensor.matmul(
                            psum,
                            x_ap(c, k, off, rl),
                            wv_sb[:, k * D + c0 : k * D + c0 + cl],
                            start=(k == 0),
                            stop=(k == KT - 1),
                        )
                    hs = c0 // DH
                    nh = cl // DH
                    out_ap = v_sb[i][
                        :rl, VB * hs : VB * (hs + nh)
                    ].rearrange("p (h c) -> p h c", c=VB)[:, :, VD : VD + DH]
                    nc.scalar.copy(
                        out=out_ap,
                        in_=psum.rearrange("p (h c) -> p h c", c=DH),
                    )

                return emit

            def vproj_units(b):
                # g0 units (heads 0-7) first so early pairs unblock sooner
                return [
                    vproj_unit(b, jt, c0, cl)
                    for c0, cl in ((0, 512), (512, 256))
                    for jt in range(2)
                ]

            # ---- out-projection units -----------------------------------
            op_cnt = [0]

            def outproj_unit(b2, m, half=None):
                def emit():
                    o0 = 0 if half is None else half * N
                    tl = N2 if half is None else N
                    t0 = b2 * N2 + o0
                    psum = ps_pj.tile([P, 512], F32, tag="pj", name="pjo")[:, :tl]
                    for k in range(KT):
                        nc.tensor.matmul(
                            psum,
                            wo_sb[:, k * D + m * P : k * D + (m + 1) * P],
                            at_sb[k][b2][:, o0 : o0 + tl],
                            start=(k == 0),
                            stop=(k == KT - 1),
                        )
                    osb = sb_osb.tile([P, 512], F32, tag="osb", name="osb")[:, :tl]
                    if op_cnt[0] % 2 == 0:
                        nc.scalar.activation(
                            osb, psum, IDENT, bias=bias_sb[:, m : m + 1]
                        )
                    else:
                        nc.vector.tensor_scalar_add(
                            out=osb, in0=psum, scalar1=bias_sb[:, m : m + 1]
                        )
                    eng = (nc.sync, nc.gpsimd)[op_cnt[0] % 2]
                    op_cnt[0] += 1
                    eng.dma_start(outT[m * P : (m + 1) * P, t0 : t0 + tl], osb)

                return emit

            # ---- one attention head-pair --------------------------------
            def pair_unit(b, p):
                def emit():
                    c = b // 2
                    tb = (b % 2) * N
                    qT = qk_sb[p][c]
                    kTt = qk_sb[NPAIR + p][c]
                    expT = []
                    for h in (0, 1):
                        e0 = DH * h
                        ps_s = ps_sc.tile([P, N2], F32, tag="sc", name="sc")
                        nc.tensor.matmul(
                            ps_s[0:P, 0:N],
                            kTt[e0 : e0 + DH, tb : tb + P],
                            qT[e0 : e0 + DH, tb : tb + N],
                            start=True,
                            stop=True,
                            tile_position=(e0, 0),
                        )
                        nc.tensor.matmul(
                            ps_s[0:JT1, N:N2],
                            kTt[e0 : e0 + DH, tb + P : tb + N],
                            qT[e0 : e0 + DH, tb : tb + N],
                            start=True,
                            stop=True,
                            tile_position=(e0, 0),
                        )
                        e = sb_exp.tile([P, N2], BF, tag="expT", name="expT")
                        nc.scalar.activation(e[:], ps_s[:], EXP)
                        expT.append(e)
                    # pso rows: 0 = denominators (ones col), 32..95 = out dims
                    pso = ps_o.tile([VB, N2], F32, tag="o", name="o")
                    v0, v1 = v_sb[2 * b], v_sb[2 * b + 1]
                    for h in (0, 1):
                        g = 2 * p + h
                        vc = VB * g
                        nc.tensor.matmul(
                            pso[:, N * h : N * h + N],
                            v0[0:P, vc : vc + VB],
                            expT[h][0:P, 0:N],
                            start=True,
                            stop=False,
                        )
                        nc.tensor.matmul(
                            pso[:, N * h : N * h + N],
                            v1[0:JT1, vc : vc + VB],
                            expT[h][0:JT1, N:N2],
                            start=False,
                            stop=True,
                        )
                    # approx reciprocal straight off psum row 0
                    rec = sb_rec.tile([1, N2], F32, tag="rec", name="rec")
                    nc.vector.reciprocal_approx_fast(out=rec[:], in_=pso[0:1, :])
                    bsb = sb_bsb.tile([DH, N2], F32, tag="bsb", name="bsb")
                    nc.gpsimd.partition_broadcast(bsb[:], rec[:])
                    for h in (0, 1):
                        nc.vector.tensor_mul(
                            out=at_sb[p][b // 2][
                                DH * h : DH * h + DH, N * (b % 2) : N * (b % 2) + N
                            ],
                            in0=pso[VD : VD + DH, N * h : N * h + N],
                            in1=bsb[:, N * h : N * h + N],
                        )

                return emit

            # ---- driver: chunk-outer pipeline ---------------------------
            # HAM discipline: attention pairs are Scalar-bound and leave the
            # PE half-idle, so never emit more than one pair between long
            # (projection) matmul units; out-projection lags 2 chunks so the
            # final drain still has long units to interleave 1:1.
            pairs_q = deque()
            for c in range(NCHUNK):
                longs = []
                for j in range(NPAIR):
                    longs.append(lambda c=c, j=j: qk_unit(c, j, 0))
                    longs.append(lambda c=c, j=j: qk_unit(c, j, 1))
                longs += vproj_units(2 * c) + vproj_units(2 * c + 1)
                if c >= 2:
                    longs += [outproj_unit(c - 2, m) for m in range(KT)]
                np_, nl = len(pairs_q), len(longs)
                done = 0
                for i, L in enumerate(longs):
                    L()
                    while pairs_q and (done + 1) * nl <= (i + 1) * np_:
                        pairs_q.popleft()()
                        done += 1
                while pairs_q:
                    pairs_q.popleft()()
                if c < NCHUNK - 1:
                    for j in range(NPAIR):
                        pairs_q.append(pair_unit(2 * c, j))
                        pairs_q.append(pair_unit(2 * c + 1, j))
            # drain: chunk-3 pairs 1:1 with outproj(2), then outproj(3) in
            # batch halves so the b=6 half interleaves with b=7 pairs
            p6 = [pair_unit(6, j) for j in range(NPAIR)]
            p7 = [pair_unit(7, j) for j in range(NPAIR)]
            for j in range(NPAIR):
                p6[j]()
                outproj_unit(2, j)()
            for j in range(NPAIR):
                p7[j]()
                outproj_unit(3, j, half=0)()
            for m in range(KT):
                outproj_unit(3, m, half=1)()

    nc.compile()
    return nc


def host_in_maps(x, w_qkv, w_out, b_out):
    """Full fp32 inputs -> list of 8 per-core input dicts (bf16, packed)."""
    bf16 = ml_dtypes.bfloat16
    wq = w_qkv[0:D] * SCALE
    wk = w_qkv[D : 2 * D]
    wv = w_qkv[2 * D : 3 * D]
    wqkT = np.concatenate([wq, wk], axis=0).T  # [768 d, 1536 e]
    # wqkP[p, j*1536 + k*256 + half*128 + col] = wqkT[k*128+p, 768*half + j*128 + col]
    wqkP = np.ascontiguousarray(
        wqkT.reshape(KT, P, 2, NPAIR, P).transpose(1, 3, 0, 2, 4).reshape(P, -1)
    ).astype(bf16)
    # w[v,o]P[p, k*768 + col] = wT[k*128+p, col]
    wvP = np.ascontiguousarray(
        wv.T.reshape(KT, P, D).transpose(1, 0, 2).reshape(P, -1)
    ).astype(bf16)
    woP = np.ascontiguousarray(
        w_out.T.reshape(KT, P, D).transpose(1, 0, 2).reshape(P, -1)
    ).astype(bf16)
    bias_ = np.ascontiguousarray(b_out.reshape(KT, P).T).astype(np.float32)
    in_maps = []
    for core in range(NCORES):
        xc = x[core * BPC : (core + 1) * BPC].reshape(T, D)
        # xP[p, c*2364 + k*394 + t'] = xc[c*394 + t', k*128 + p]
        xP = np.ascontiguousarray(
            xc.reshape(NCHUNK, N2, KT, P).transpose(3, 0, 2, 1).reshape(P, -1)
        ).astype(bf16)
        in_maps.append(
            {"xP": xP, "wqkP": wqkP, "wvP": wvP, "woP": woP, "bias": bias_}
        )
    return in_maps


def host_gather(results):
    """8 per-core {outT: [768, 1576] fp32} -> full [64, 197, 768] fp32."""
    out = np.empty((B, N, D), dtype=np.float32)
    for c in range(NCORES):
        oc = results[c]["outT"]  # [D, T]
        out[c * BPC : (c + 1) * BPC] = oc.T.reshape(BPC, N, D)
    return out


_NC_CACHE = []


def kernel(x, w_qkv, w_out, b_out):
    """Full-input entry point: shards batch over 8 NeuronCores, runs the
    Bass kernel, gathers the full [64, 197, 768] fp32 output."""
    if not _NC_CACHE:
        _NC_CACHE.append(build_nc())
    nc = _NC_CACHE[0]
    in_maps = host_in_maps(
        np.asarray(x, dtype=np.float32),
        np.asarray(w_qkv, dtype=np.float32),
        np.asarray(w_out, dtype=np.float32),
        np.asarray(b_out, dtype=np.float32),
    )
    res = run_bass_kernel_spmd(nc, in_maps, core_ids=list(range(NCORES)))
    return host_gather(res.results)
